# revision 3
# baseline (speedup 1.0000x reference)
"""Trainium2 Bass kernel v2 for GQA attention block (B=2, S=2048, H=2048,
16 q-heads / 4 kv-heads, head_dim=128, RoPE, causal) on 8 NeuronCores.

Sharding: core c -> batch b = c // 4, kv-group g = c % 4
  (q heads 4g..4g+3, kv head g).  Each core computes its batch's
  attention for its 4 query heads plus the partial output projection
  over its 512 hidden columns of w_o; host sums the 4 partials per batch.

v2 changes vs baseline:
  - all heavy streams in bf16 (hT, weights, qk/v, e, attnT, out partials):
    halves DMA bytes (fixes phase-1 DMA starvation) and SBUF footprint
  - wqk packed [KT,128,768] so each kt is ONE full-rate DMA
  - PSUM->SBUF copies moved off DVE: qkv/v copies on ACT, o-proj copies
    split DVE/Pool
  - RoPE cos-mul reads the PSUM tile directly (no extra raw copy user)
  - wo/masks DMAs issued inside phase 1 so phase 2/3 never waits
"""

import contextlib
import math
import numpy as np
import ml_dtypes

import concourse.bacc as bacc
import concourse.mybir as mybir
import concourse.tile as tile
from concourse.bass_utils import run_bass_kernel_spmd
from concourse.masks import make_identity

F32 = mybir.dt.float32
BF16 = mybir.dt.bfloat16
AF = mybir.ActivationFunctionType
NPBF = ml_dtypes.bfloat16

S = 2048
H = 2048
D = 128            # head dim
KT = 16            # contraction tiles over hidden (2048/128)
QW = 512           # s-quarter width in the projection phase
NQ = 512           # query block width in attention
NUM_Q_LOCAL = 4    # q heads per core
SCALE = 1.0 / math.sqrt(D)
NEG = -1.0e9

_CACHED = {}


def build_nc(loop_n=None):
    nc = bacc.Bacc(None, target_bir_lowering=False)
    hT = nc.dram_tensor("hT", [H, S], BF16, kind="ExternalInput")
    wqk = nc.dram_tensor("wqk", [KT, 128, 6 * 128], BF16, kind="ExternalInput")
    cosT = nc.dram_tensor("cosT", [D, S], BF16, kind="ExternalInput")
    sinT = nc.dram_tensor("sinT", [D, S], BF16, kind="ExternalInput")
    wo = nc.dram_tensor("wo", [4, 128, H], BF16, kind="ExternalInput")
    out = nc.dram_tensor("out", [S, H], BF16, kind="ExternalOutput")

    with tile.TileContext(nc) as tc:
        with tc.tile_pool(name="persist", bufs=1) as pp:
          with (tc.For_i(0, loop_n, 1) if loop_n else contextlib.nullcontext()):
            # ---- persistent tiles (live across phases) ----
            qk = [pp.tile([128, S], BF16, name=f"qk{i}", tag=f"qk{i}") for i in range(5)]
            v_sb = pp.tile([128, S], BF16, tag="v")
            cos_sb = pp.tile([128, S], BF16, tag="cos")
            sin_sb = pp.tile([128, S], BF16, tag="sin")
            ones_r = pp.tile([128, 1], BF16, tag="onesr")
            ident = pp.tile([128, 128], BF16, tag="ident")
            tri = pp.tile([128, 128], BF16, tag="tri")
            wo_sb = [
                pp.tile([128, H], BF16, name=f"wo{kb}", tag=f"wo{kb}")
                for kb in range(4)
            ]

            # ---- Phase 1: fused QKV projection (s-quarters, resident weights)
            # + RoPE and v-transpose interleaved per quarter ----
            NQT = S // QW
            with (
                tc.tile_pool(name="ht", bufs=1) as htp,
                tc.tile_pool(name="wq", bufs=1) as wqp,
                tc.tile_pool(name="vtp", bufs=1) as vtp,
                tc.tile_pool(name="rope", bufs=3) as rp,
                tc.tile_pool(name="psq", bufs=6, space="PSUM") as psq,
                tc.tile_pool(name="psv", bufs=2, space="PSUM") as psv,
            ):
                vT_sb = vtp.tile([128, S], BF16, tag="vT")
                w_all = wqp.tile([128, KT * 768], BF16, tag="wall")
                # hT resident as two column-halves per kt: [128,1024] rows
                # are 2KB -> full DMA rate; q0 paced by (w+htA) per kt
                htA = [htp.tile([128, 1024], BF16, name=f"htA{kt}", tag=f"htA{kt}")
                       for kt in range(KT)]
                htB = [htp.tile([128, 1024], BF16, name=f"htB{kt}", tag=f"htB{kt}")
                       for kt in range(KT)]
                for kt in range(KT):
                    nc.sync.dma_start(
                        out=w_all[:, kt * 768 : (kt + 1) * 768], in_=wqk[kt]
                    )
                    nc.sync.dma_start(
                        out=htA[kt][:], in_=hT[kt * 128 : (kt + 1) * 128, 0:1024]
                    )
                for kt in range(KT):
                    nc.sync.dma_start(
                        out=htB[kt][:], in_=hT[kt * 128 : (kt + 1) * 128, 1024:2048]
                    )
                nc.sync.dma_start(out=cos_sb[:], in_=cosT[:])
                nc.sync.dma_start(out=sin_sb[:], in_=sinT[:])
                nc.vector.memset(ones_r[:], 1.0)
                make_identity(nc, ident[:])
                # tri[sk, y] = 1 if y >= sk else 0 (causal keep mask)
                nc.gpsimd.memset(tri[:], 1.0)
                nc.gpsimd.affine_select(
                    out=tri[:],
                    in_=tri[:],
                    compare_op=mybir.AluOpType.is_ge,
                    fill=0.0,
                    base=0,
                    pattern=[[1, 128]],
                    channel_multiplier=-1,
                )
                for kb in range(4):
                    nc.sync.dma_start(out=wo_sb[kb][:], in_=wo[kb])
                for q in range(NQT):
                    s0 = q * QW
                    half, hs0 = (htA, s0) if q < 2 else (htB, s0 - 1024)
                    for i in range(6):
                        ps = psq.tile([128, QW], F32, tag="psq")
                        for kt in range(KT):
                            nc.tensor.matmul(
                                ps[:],
                                lhsT=w_all[:, kt * 768 + i * 128 : kt * 768 + (i + 1) * 128],
                                rhs=half[kt][:, hs0 : hs0 + QW],
                                start=(kt == 0),
                                stop=(kt == KT - 1),
                            )
                        if i < 5:
                            # ACT copy releases the PSUM chain immediately;
                            # RoPE then runs DVE-only off the SBUF copy
                            nc.scalar.copy(qk[i][:, s0 : s0 + QW], ps[:])
                            rot = rp.tile([128, QW], BF16, tag="rot")
                            rot2 = rp.tile([128, QW], BF16, tag="rot2")
                            tmp = rp.tile([128, QW], BF16, tag="tmp")
                            nc.sync.dma_start(
                                out=rot[0:64, :], in_=qk[i][64:128, s0 : s0 + QW]
                            )
                            nc.sync.dma_start(
                                out=rot[64:128, :], in_=qk[i][0:64, s0 : s0 + QW]
                            )
                            nc.vector.tensor_mul(
                                rot2[:], qk[i][:, s0 : s0 + QW], cos_sb[:, s0 : s0 + QW]
                            )
                            nc.vector.tensor_mul(
                                tmp[:], rot[:], sin_sb[:, s0 : s0 + QW]
                            )
                            nc.vector.tensor_add(
                                qk[i][:, s0 : s0 + QW], rot2[:], tmp[:]
                            )
                        else:
                            nc.scalar.copy(vT_sb[:, s0 : s0 + QW], ps[:])
                            for sbl in range(QW // 128):
                                sb = q * (QW // 128) + sbl
                                psvt = psv.tile([128, 128], BF16, tag="psv")
                                nc.tensor.transpose(
                                    psvt[:],
                                    vT_sb[:, sb * 128 : (sb + 1) * 128],
                                    ident[:],
                                )
                                nc.vector.tensor_copy(
                                    v_sb[:, sb * 128 : (sb + 1) * 128], psvt[:]
                                )

            # ---- Phase 2 + 3 interleaved per query block ----
            with (
                tc.tile_pool(name="attn", bufs=1) as ap,
                tc.tile_pool(name="epool", bufs=4) as ep,
                tc.tile_pool(name="small", bufs=4) as sp,
                tc.tile_pool(name="obuf", bufs=4) as ob,
                tc.tile_pool(name="pss", bufs=4, space="PSUM") as pss,
                tc.tile_pool(name="pspv", bufs=2, space="PSUM") as pspv,
                tc.tile_pool(name="psden", bufs=2, space="PSUM") as psden,
            ):
                attnT = [
                    ap.tile([128, S], BF16, name=f"at{h}", tag=f"at{h}")
                    for h in range(4)
                ]

                kT = qk[4]
                for qb in range(S // NQ):
                    q0 = qb * NQ
                    nj = 4 * qb + 4
                    for h in range(NUM_Q_LOCAL):
                        qT = qk[h]
                        pv = pspv.tile([128, NQ], F32, tag="pv")
                        den = psden.tile([1, NQ], F32, tag="den")
                        for j in range(nj):
                            r4 = j - 4 * qb
                            # diagonal blocks: columns sq < r4*128 are fully
                            # masked -> narrow the whole j-chain to [off:NQ)
                            off = max(0, r4) * 128
                            sps = pss.tile([128, NQ], F32, tag="sc")
                            nc.tensor.matmul(
                                sps[:, off:NQ],
                                lhsT=kT[:, j * 128 : (j + 1) * 128],
                                rhs=qT[:, q0 + off : q0 + NQ],
                                start=True,
                                stop=True,
                            )
                            e = ep.tile([128, NQ], BF16, tag="e")
                            nc.scalar.activation(
                                e[:, off:NQ], sps[:, off:NQ], AF.Exp, scale=SCALE
                            )
                            if r4 >= 0:
                                # causal zeroing of the 128-wide diagonal
                                # sub-block via precomputed triangle mask
                                nc.vector.tensor_mul(
                                    e[:, off : off + 128],
                                    e[:, off : off + 128],
                                    tri[:],
                                )
                            nc.tensor.matmul(
                                pv[:, off:NQ],
                                lhsT=v_sb[:, j * 128 : (j + 1) * 128],
                                rhs=e[:, off:NQ],
                                start=(j == 0),
                                stop=(j == nj - 1),
                            )
                            nc.tensor.matmul(
                                den[:, off:NQ],
                                lhsT=ones_r[:],
                                rhs=e[:, off:NQ],
                                start=(j == 0),
                                stop=(j == nj - 1),
                            )
                        rec = sp.tile([1, NQ], F32, tag="rec")
                        nc.vector.reciprocal(rec[:], den[:])
                        bcs = sp.tile([128, NQ], F32, tag="bcs")
                        nc.gpsimd.partition_broadcast(bcs[:], rec[:])
                        nc.vector.tensor_mul(
                            attnT[h][:, q0 : q0 + NQ], pv[:], bcs[:]
                        )
                    # o-projection for the 4 s-blocks of this query block;
                    # one full-width (4KB-row) output DMA per s-block
                    for sbl in range(NQ // 128):
                        sb = qb * 4 + sbl
                        osb = ob.tile([128, H], BF16, tag="osb")
                        for n in range(H // NQ):
                            pst = pss.tile([128, NQ], F32, tag="sc")
                            for kb in range(4):
                                nc.tensor.matmul(
                                    pst[:],
                                    lhsT=attnT[kb][:, sb * 128 : (sb + 1) * 128],
                                    rhs=wo_sb[kb][:, n * NQ : (n + 1) * NQ],
                                    start=(kb == 0),
                                    stop=(kb == 3),
                                )
                            if n % 2 == 0:
                                nc.vector.tensor_copy(
                                    osb[:, n * NQ : (n + 1) * NQ], pst[:]
                                )
                            else:
                                nc.scalar.copy(
                                    osb[:, n * NQ : (n + 1) * NQ], pst[:]
                                )
                        nc.scalar.dma_start(
                            out=out[sb * 128 : (sb + 1) * 128, :],
                            in_=osb[:],
                        )

    nc.compile()
    return nc


def _prep_inputs(hidden_states, cos, sin, w_qkv, w_o):
    """Build the 8 per-core input maps (host-side shard + transpose + bf16)."""
    hidden_states = np.asarray(hidden_states, dtype=np.float32)
    cos = np.asarray(cos, dtype=np.float32)
    sin = np.asarray(sin, dtype=np.float32)
    w_qkv = np.asarray(w_qkv, dtype=np.float32)
    w_o = np.asarray(w_o, dtype=np.float32)

    cosT = np.ascontiguousarray(cos.T).astype(NPBF)
    sinT_f = np.ascontiguousarray(sin.T).copy()
    sinT_f[0:64] *= -1.0  # rotate_half sign folded into sin
    sinT = sinT_f.astype(NPBF)

    hT = [np.ascontiguousarray(hidden_states[b].T).astype(NPBF) for b in range(2)]

    in_maps = []
    for c in range(8):
        b, g = divmod(c, 4)
        W6 = np.stack(
            [w_qkv[(4 * g + i) * 128 : (4 * g + i + 1) * 128] for i in range(4)]
            + [w_qkv[(16 + g) * 128 : (17 + g) * 128]]
            + [w_qkv[(20 + g) * 128 : (21 + g) * 128]]
        )  # [6, 128 m, 2048 h]
        # -> [kt, 128 h, 6*128 m]: per-kt one contiguous full-rate DMA
        wqk_pack = np.ascontiguousarray(
            W6.transpose(2, 0, 1).reshape(KT, 128, 6, 128).reshape(KT, 128, 768)
        ).astype(NPBF)
        wo_pack = np.ascontiguousarray(
            np.stack(
                [
                    w_o[:, (4 * g + kb) * 128 : (4 * g + kb + 1) * 128].T
                    for kb in range(4)
                ]
            )
        ).astype(NPBF)  # [4, 128 hd, 2048 o]
        in_maps.append(
            dict(
                hT=hT[b],
                wqk=wqk_pack,
                cosT=cosT,
                sinT=sinT,
                wo=wo_pack,
            )
        )
    return in_maps


def run(hidden_states, cos, sin, w_qkv, w_o, trace=False, **trace_kwargs):
    if "nc" not in _CACHED:
        _CACHED["nc"] = build_nc()
    nc = _CACHED["nc"]
    in_maps = _prep_inputs(hidden_states, cos, sin, w_qkv, w_o)
    res = run_bass_kernel_spmd(
        nc, in_maps, core_ids=list(range(8)), trace=trace, **trace_kwargs
    )
    outs = [res.results[c]["out"].astype(np.float32) for c in range(8)]
    full = np.stack(
        [
            outs[0] + outs[1] + outs[2] + outs[3],
            outs[4] + outs[5] + outs[6] + outs[7],
        ]
    ).astype(np.float32)
    return full, res


def kernel(hidden_states, cos, sin, w_qkv, w_o):
    full, _ = run(hidden_states, cos, sin, w_qkv, w_o, trace=False)
    return full
